# revision 12
# baseline (speedup 1.0000x reference)
"""Trainium2 Bass kernel for ContextQueryAttention (BiDAF-style), v5.

Math (per batch):
    S[n,m] = c@w0 [n] + (q@w1 + bias)[m] + sum_d c[n,d]*wm[d]*q[m,d]
    S_  = softmax_m(S + MASK*(1-q_mask))          # row softmax
    S_T = softmax_n(S + MASK*(1-c_mask)).T        # col softmax, transposed
    c2q = S_ @ q ;  q2c = S_ @ (S_T @ c)
    out = [c | c2q | c*c2q | c*q2c]

Factorization: with G = exp(sub2), A = exp(sub0), B = exp(sub1+bias),
    S_[n,m]  = G[n,m]*Bq[m] / (G @ Bq)[n]         Bq = B * q_mask
    S_T[m,n] = G[n,m]*Ac[n] / (G.T @ Ac)[m]       Ac = A * c_mask

v5 (vs v4, 91 us):
  - Gn via xbar DMA-transpose of GT (sync queue) instead of PE
    transposes: PE drops from 36 to 28 matmuls/batch (GN_VIA switch
    falls back to the PE path).
  - device output is UNNORMALIZED [c2q*den | den-col | q2c*den]; the
    row denominator rides along and the host divides, removing the 8
    per-tile reciprocal ops; evacuation is one strided plain copy/tile.
  - per-batch staggered loads (cT/qside per batch) instead of one 4 MB
    up-front global, so the pipeline fills in ~2 us instead of ~14 us.
  - output stores issue from the scalar queue (separate HWDGE ring from
    the loads).
Sharding: data-parallel over batch, 8 batches per core on 8 cores.
"""

import sys

if "/opt/trn_rl_repo" not in sys.path:
    sys.path.insert(0, "/opt/trn_rl_repo")

import numpy as np
import ml_dtypes

import concourse.bass as bass
import concourse.mybir as mybir
import concourse.tile as tile
from concourse import bacc
from concourse.bass_utils import run_bass_kernel_spmd
from concourse.masks import make_identity

B, N, M, D = 64, 1024, 128, 256
NCORES = 8
BPC = B // NCORES  # batches per core
NT = N // 128      # n-tiles per batch
DT = D // 128      # d-tiles

F32 = mybir.dt.float32
BF16 = mybir.dt.bfloat16
EXP = mybir.ActivationFunctionType.Exp
X = mybir.AxisListType.X
BF = ml_dtypes.bfloat16

GN_VIA = "snat"  # "dma" | "pe" | "snat" (recompute S in natural layout)


def _build(bpc: int = BPC):
    nc = bacc.Bacc(trn_type="TRN2")

    # all staged by the host in device/SBUF layout so every DMA is a
    # contiguous <=3-dim AP with multi-KB per-partition lines
    c_d = nc.dram_tensor("c", [bpc, 128, NT, D + 1], BF16, kind="ExternalInput")
    ct_d = nc.dram_tensor("ct", [bpc, 128, DT, N], BF16, kind="ExternalInput")
    # qside: cols [0:256] = (q*wm)^T as [DT,128], cols [256:513] = [q*Bq | Bq]
    qs_d = nc.dram_tensor("qs", [bpc, 128, 513], BF16, kind="ExternalInput")
    # lacbq: [:, b, 0:NT] = c@w0 + log(c_mask); [:, b, NT] = Bq
    lb_d = nc.dram_tensor("lb", [128, bpc, NT + 1], F32, kind="ExternalInput")
    o_d = nc.dram_tensor("out", [bpc, 128, NT, 2 * (D + 1)], BF16, kind="ExternalOutput")

    with tile.TileContext(nc) as tc:
        with (
            tc.tile_pool(name="glob", bufs=1) as gp,
            tc.tile_pool(name="pa", bufs=3) as pa,
            tc.tile_pool(name="pb", bufs=2) as pb,
            tc.tile_pool(name="po", bufs=2) as po,
            tc.tile_pool(name="ps_mm", bufs=3, space="PSUM") as ps_mm,
            tc.tile_pool(name="ps_t", bufs=1, space="PSUM") as ps_t,
            tc.tile_pool(name="ps_o", bufs=2, space="PSUM") as ps_o,
        ):
            # ---- globals ----
            if GN_VIA == "pe":
                ident = gp.tile([128, 128], BF16)
                make_identity(nc, ident)
            lb_all = gp.tile([128, bpc, NT + 1], F32)
            nc.sync.dma_start(out=lb_all, in_=lb_d[:, :, :])

            def stage_a1(b):
                """loads + S^T matmuls + GT exps + Ac exp."""
                st = {}
                c_n1 = pa.tile([128, NT, D + 1], BF16, tag="c_n1")
                nc.sync.dma_start(out=c_n1, in_=c_d[b])
                cTb = pa.tile([128, DT, N], BF16, tag="cTb")
                nc.sync.dma_start(out=cTb, in_=ct_d[b])
                qside = pa.tile([128, 513], BF16, tag="qside")
                nc.sync.dma_start(out=qside, in_=qs_d[b])

                GT = pa.tile([128, N], BF16, tag="GT")
                for h in range(2):
                    stp = ps_mm.tile([128, 512], F32, tag="mm")
                    for j in range(DT):
                        nc.tensor.matmul(
                            stp,
                            qside[:, 128 * j : 128 * (j + 1)],
                            cTb[:, j, 512 * h : 512 * (h + 1)],
                            start=(j == 0),
                            stop=(j == DT - 1),
                        )
                    nc.scalar.activation(GT[:, 512 * h : 512 * (h + 1)], stp, EXP)

                if GN_VIA != "snat":
                    ac = pa.tile([128, NT], F32, tag="ac")
                    nc.scalar.activation(ac, lb_all[:, b, 0:NT], EXP)
                    st["ac"] = ac
                st["c_n1"], st["GT"] = c_n1, GT
                st["qside"], st["cTb"] = qside, cTb
                st["qbx"] = qside[:, 256:513]
                return st

            def stage_a2(b, st):
                """Gn' = G * Ac in n-partition layout."""
                Gn = pa.tile([128, NT, M], BF16, tag="Gn")
                if GN_VIA == "snat":
                    # recompute S with n as the output partition; exp folds
                    # Ac in via the per-partition bias lac = c@w0 + log(cm)
                    qside, cTb = st["qside"], st["cTb"]
                    for g in range(2):
                        sn = ps_mm.tile([128, 512], F32, tag="mm")
                        for u in range(4):
                            i = 4 * g + u
                            for j in range(DT):
                                nc.tensor.matmul(
                                    sn[:, 128 * u : 128 * (u + 1)],
                                    cTb[:, j, 128 * i : 128 * (i + 1)],
                                    qside[:, 128 * j : 128 * (j + 1)],
                                    start=(j == 0),
                                    stop=(j == DT - 1),
                                )
                        for u in range(4):
                            i = 4 * g + u
                            nc.scalar.activation(
                                Gn[:, i, :], sn[:, 128 * u : 128 * (u + 1)], EXP,
                                bias=lb_all[:, b, i : i + 1], scale=1.0,
                            )
                    st["Gn"] = Gn
                    return
                ac = st["ac"]
                if GN_VIA == "dma":
                    gnu = pa.tile([128, NT, M], BF16, tag="gnu")
                    for i in range(NT):
                        nc.sync.dma_start(
                            out=gnu[:, i, :],
                            in_=st["GT"][:, 128 * i : 128 * (i + 1)],
                            transpose=True,
                        )
                    for i in range(NT):
                        nc.vector.tensor_scalar_mul(
                            out=Gn[:, i, :], in0=gnu[:, i, :], scalar1=ac[:, i : i + 1]
                        )
                else:
                    trp = ps_mm.tile([128, NT, 128], BF16, tag="mm")
                    for i in range(NT):
                        nc.tensor.transpose(
                            trp[:, i, :], st["GT"][:, 128 * i : 128 * (i + 1)], ident
                        )
                    for i in range(NT):
                        if i % 2 == 0:
                            nc.scalar.mul(Gn[:, i, :], trp[:, i, :], ac[:, i : i + 1])
                        else:
                            nc.vector.tensor_scalar_mul(
                                out=Gn[:, i, :], in0=trp[:, i, :],
                                scalar1=ac[:, i : i + 1],
                            )
                st["Gn"] = Gn

            def stage_b(b, st):
                """t = S_T' @ [c | 1] (numerator + col-sum column) -> tB."""
                tps = ps_t.tile([128, D + 1], F32, tag="tps")
                for i in range(NT):
                    nc.tensor.matmul(
                        tps, st["Gn"][:, i, :], st["c_n1"][:, i, :],
                        start=(i == 0), stop=(i == NT - 1),
                    )
                csi = pb.tile([128, 1], F32, tag="csi")
                nc.vector.reciprocal(csi, tps[:, D : D + 1])
                bqc = pb.tile([128, 1], F32, tag="bqc")
                nc.vector.tensor_mul(bqc, lb_all[:, b, NT : NT + 1], csi)
                tB = pb.tile([128, D + 1], BF16, tag="tB")
                nc.scalar.mul(tB, tps[:, 0 : D + 1], bqc)
                st["tB"] = tB

            def stage_c(b, st):
                """c2q/q2c matmuls + strided evacuation + store.
                Output stays denominator-scaled; host divides."""
                obuf = po.tile([128, NT, 2 * (D + 1)], BF16, tag="obuf")
                for i in range(NT):
                    big2 = ps_o.tile([128, 1024], F32, tag="big2")
                    gsl = st["GT"][:, 128 * i : 128 * (i + 1)]
                    nc.tensor.matmul(
                        big2[:, 0 : D + 1], gsl, st["qbx"], start=True, stop=True
                    )
                    nc.tensor.matmul(
                        big2[:, 512 : 512 + D + 1], gsl, st["tB"], start=True, stop=True
                    )
                    src = big2.rearrange("p (g x) -> p g x", g=2)[:, :, 0 : D + 1]
                    dst = obuf[:, i, :].rearrange("p (g x) -> p g x", g=2)
                    if GN_VIA != "snat" and i % 2 == 0:
                        nc.scalar.copy(dst, src)
                    else:
                        nc.vector.tensor_copy(dst, src)
                nc.scalar.dma_start(out=o_d[b], in_=obuf)

            # software pipeline, stage_a1 runs 2 batches ahead:
            #   iter b: a1(b+2) | a2(b+1) | t(b) + out(b)
            sts = {}
            sts[0] = stage_a1(0)
            if bpc > 1:
                sts[1] = stage_a1(1)
            stage_a2(0, sts[0])
            for b in range(bpc):
                if b + 2 < bpc:
                    sts[b + 2] = stage_a1(b + 2)
                if b + 1 < bpc:
                    stage_a2(b + 1, sts[b + 1])
                stage_b(b, sts[b])
                stage_c(b, sts[b])
                del sts[b]

    nc.finalize()
    return nc


_NC = None


def _get_nc():
    global _NC
    if _NC is None:
        _NC = _build()
    return _NC


def _in_maps(inputs):
    """Host-side staging: shard over batch, cast to bf16, pre-stage each
    tensor in the exact SBUF layout the kernel DMAs it into, and
    precompute the per-row/per-col softmax scale vectors."""
    c = np.ascontiguousarray(inputs["c"], dtype=np.float32)
    q = np.ascontiguousarray(inputs["q"], dtype=np.float32)
    w0 = np.asarray(inputs["w0"], dtype=np.float32).reshape(D)
    w1 = np.asarray(inputs["w1"], dtype=np.float32).reshape(D)
    wm = np.asarray(inputs["wm"], dtype=np.float32).reshape(D)
    bias = np.asarray(inputs["bias"], dtype=np.float32).reshape(M)
    cmf = np.asarray(inputs["c_mask"], dtype=np.float32)  # [B, N]
    qmf = np.asarray(inputs["q_mask"], dtype=np.float32)  # [B, M]

    cb = c.astype(BF)
    # c natural with ones column: [B, 128, NT, D+1]
    c_st = np.ones((B, 128, NT, D + 1), dtype=BF)
    c_st[:, :, :, 0:D] = cb.reshape(B, NT, 128, D).transpose(0, 2, 1, 3)
    # c^T: [B, 128, DT, N]
    ct_st = np.ascontiguousarray(
        cb.transpose(2, 0, 1).reshape(DT, 128, B, N).transpose(2, 1, 0, 3)
    )
    # qside: [B, 128, 513] = [(q*wm)^T as DTx128 | q*Bq | Bq]
    bq = np.exp(q @ w1 + bias[None, :]) * qmf  # [B, M]
    qs_st = np.empty((B, 128, 513), dtype=BF)
    qwm = (q * wm[None, None, :]).astype(BF)  # [B, M, D]
    qs_st[:, :, 0:256] = (
        qwm.transpose(2, 0, 1).reshape(DT, 128, B, M).transpose(2, 1, 0, 3)
        .reshape(B, 128, 256)
    )
    qs_st[:, :, 256:512] = (q * bq[:, :, None]).astype(BF)
    qs_st[:, :, 512] = bq.astype(BF)
    # lacbq: [128, B, NT+1]
    lb_st = np.empty((128, B, NT + 1), dtype=np.float32)
    lac = c @ w0 + np.where(cmf > 0.5, 0.0, -1e4)  # [B, N]
    lb_st[:, :, 0:NT] = lac.reshape(B, NT, 128).transpose(2, 0, 1)
    lb_st[:, :, NT] = bq.transpose(1, 0)

    in_maps = []
    for k in range(NCORES):
        s = slice(k * BPC, (k + 1) * BPC)
        in_maps.append(
            {
                "c": np.ascontiguousarray(c_st[s]),
                "ct": np.ascontiguousarray(ct_st[s]),
                "qs": np.ascontiguousarray(qs_st[s]),
                "lb": np.ascontiguousarray(lb_st[:, s]),
            }
        )
    return in_maps


def kernel(c, q, c_mask, q_mask, w0, w1, wm, bias):
    inputs = {
        "c": c, "q": q, "c_mask": c_mask, "q_mask": q_mask,
        "w0": w0, "w1": w1, "wm": wm, "bias": bias,
    }
    in_maps = _in_maps(inputs)
    res = run_bass_kernel_spmd(_get_nc(), in_maps, core_ids=list(range(NCORES)))
    dev = np.concatenate(
        [np.asarray(res.results[k]["out"]) for k in range(NCORES)], axis=0
    )  # [B, 128, NT, 2*(D+1)]
    dev = dev.transpose(0, 2, 1, 3).reshape(B, N, 2 * (D + 1)).astype(np.float32)
    den = dev[:, :, D : D + 1]  # (G @ Bq)[n], denominator-scaled path
    c2q = dev[:, :, 0:D] / den
    q2c = dev[:, :, D + 1 : 2 * D + 1] / den
    c32 = np.ascontiguousarray(c, dtype=np.float32)
    out = np.empty((B, N, 4 * D), dtype=np.float32)
    out[:, :, 0:D] = c32
    out[:, :, D : 2 * D] = c2q
    out[:, :, 2 * D : 3 * D] = c32 * c2q
    out[:, :, 3 * D : 4 * D] = c32 * q2c
    return out


# revision 13
# speedup vs baseline: 1.3241x; 1.3241x over previous
"""Trainium2 Bass kernel for ContextQueryAttention (BiDAF-style), v5.

Math (per batch):
    S[n,m] = c@w0 [n] + (q@w1 + bias)[m] + sum_d c[n,d]*wm[d]*q[m,d]
    S_  = softmax_m(S + MASK*(1-q_mask))          # row softmax
    S_T = softmax_n(S + MASK*(1-c_mask)).T        # col softmax, transposed
    c2q = S_ @ q ;  q2c = S_ @ (S_T @ c)
    out = [c | c2q | c*c2q | c*q2c]

Factorization: with G = exp(sub2), A = exp(sub0), B = exp(sub1+bias),
    S_[n,m]  = G[n,m]*Bq[m] / (G @ Bq)[n]         Bq = B * q_mask
    S_T[m,n] = G[n,m]*Ac[n] / (G.T @ Ac)[m]       Ac = A * c_mask

v5 (vs v4, 91 us):
  - Gn via xbar DMA-transpose of GT (sync queue) instead of PE
    transposes: PE drops from 36 to 28 matmuls/batch (GN_VIA switch
    falls back to the PE path).
  - device output is UNNORMALIZED [c2q*den | den-col | q2c*den]; the
    row denominator rides along and the host divides, removing the 8
    per-tile reciprocal ops; evacuation is one strided plain copy/tile.
  - per-batch staggered loads (cT/qside per batch) instead of one 4 MB
    up-front global, so the pipeline fills in ~2 us instead of ~14 us.
  - output stores issue from the scalar queue (separate HWDGE ring from
    the loads).
Sharding: data-parallel over batch, 8 batches per core on 8 cores.
"""

import sys

if "/opt/trn_rl_repo" not in sys.path:
    sys.path.insert(0, "/opt/trn_rl_repo")

import numpy as np
import ml_dtypes

import concourse.bass as bass
import concourse.mybir as mybir
import concourse.tile as tile
from concourse import bacc
from concourse.bass_utils import run_bass_kernel_spmd
from concourse.masks import make_identity

B, N, M, D = 64, 1024, 128, 256
NCORES = 8
BPC = B // NCORES  # batches per core
NT = N // 128      # n-tiles per batch
DT = D // 128      # d-tiles

F32 = mybir.dt.float32
BF16 = mybir.dt.bfloat16
EXP = mybir.ActivationFunctionType.Exp
X = mybir.AxisListType.X
BF = ml_dtypes.bfloat16

GN_VIA = "pe"  # "dma" | "pe" | "snat" (recompute S in natural layout)


def _build(bpc: int = BPC):
    nc = bacc.Bacc(trn_type="TRN2")

    # all staged by the host in device/SBUF layout so every DMA is a
    # contiguous <=3-dim AP with multi-KB per-partition lines
    c_d = nc.dram_tensor("c", [bpc, 128, NT, D + 1], BF16, kind="ExternalInput")
    ct_d = nc.dram_tensor("ct", [bpc, 128, DT, N], BF16, kind="ExternalInput")
    # qside: cols [0:256] = (q*wm)^T as [DT,128], cols [256:513] = [q*Bq | Bq]
    qs_d = nc.dram_tensor("qs", [bpc, 128, 513], BF16, kind="ExternalInput")
    # lacbq: [:, b, 0:NT] = c@w0 + log(c_mask); [:, b, NT] = Bq
    lb_d = nc.dram_tensor("lb", [128, bpc, NT + 1], F32, kind="ExternalInput")
    o_d = nc.dram_tensor("out", [bpc, 128, NT, 2 * (D + 1)], BF16, kind="ExternalOutput")

    with tile.TileContext(nc) as tc:
        with (
            tc.tile_pool(name="glob", bufs=1) as gp,
            tc.tile_pool(name="pa", bufs=3) as pa,
            tc.tile_pool(name="pb", bufs=2) as pb,
            tc.tile_pool(name="po", bufs=2) as po,
            tc.tile_pool(name="ps_mm", bufs=3, space="PSUM") as ps_mm,
            tc.tile_pool(name="ps_t", bufs=1, space="PSUM") as ps_t,
            tc.tile_pool(name="ps_o", bufs=2, space="PSUM") as ps_o,
        ):
            # ---- globals ----
            if GN_VIA == "pe":
                ident = gp.tile([128, 128], BF16)
                make_identity(nc, ident)
            lb_all = gp.tile([128, bpc, NT + 1], F32)
            nc.sync.dma_start(out=lb_all, in_=lb_d[:, :, :])

            def stage_a1(b):
                """loads + S^T matmuls + GT exps + Ac exp."""
                st = {}
                c_n1 = pa.tile([128, NT, D + 1], BF16, tag="c_n1")
                nc.sync.dma_start(out=c_n1, in_=c_d[b])
                cTb = pa.tile([128, DT, N], BF16, tag="cTb")
                nc.sync.dma_start(out=cTb, in_=ct_d[b])
                qside = pa.tile([128, 513], BF16, tag="qside")
                nc.sync.dma_start(out=qside, in_=qs_d[b])

                GT = pa.tile([128, N], BF16, tag="GT")
                for h in range(2):
                    stp = ps_mm.tile([128, 512], F32, tag="mm")
                    for j in range(DT):
                        nc.tensor.matmul(
                            stp,
                            qside[:, 128 * j : 128 * (j + 1)],
                            cTb[:, j, 512 * h : 512 * (h + 1)],
                            start=(j == 0),
                            stop=(j == DT - 1),
                        )
                    nc.scalar.activation(GT[:, 512 * h : 512 * (h + 1)], stp, EXP)

                if GN_VIA != "snat":
                    ac = pa.tile([128, NT], F32, tag="ac")
                    nc.scalar.activation(ac, lb_all[:, b, 0:NT], EXP)
                    st["ac"] = ac
                st["c_n1"], st["GT"] = c_n1, GT
                st["qside"], st["cTb"] = qside, cTb
                st["qbx"] = qside[:, 256:513]
                return st

            def stage_a2(b, st):
                """Gn' = G * Ac in n-partition layout."""
                Gn = pa.tile([128, NT, M], BF16, tag="Gn")
                if GN_VIA == "snat":
                    # recompute S with n as the output partition; exp folds
                    # Ac in via the per-partition bias lac = c@w0 + log(cm)
                    qside, cTb = st["qside"], st["cTb"]
                    for g in range(2):
                        sn = ps_mm.tile([128, 512], F32, tag="mm")
                        for u in range(4):
                            i = 4 * g + u
                            for j in range(DT):
                                nc.tensor.matmul(
                                    sn[:, 128 * u : 128 * (u + 1)],
                                    cTb[:, j, 128 * i : 128 * (i + 1)],
                                    qside[:, 128 * j : 128 * (j + 1)],
                                    start=(j == 0),
                                    stop=(j == DT - 1),
                                )
                        for u in range(4):
                            i = 4 * g + u
                            nc.scalar.activation(
                                Gn[:, i, :], sn[:, 128 * u : 128 * (u + 1)], EXP,
                                bias=lb_all[:, b, i : i + 1], scale=1.0,
                            )
                    st["Gn"] = Gn
                    return
                ac = st["ac"]
                if GN_VIA == "dma":
                    gnu = pa.tile([128, NT, M], BF16, tag="gnu")
                    for i in range(NT):
                        nc.sync.dma_start(
                            out=gnu[:, i, :],
                            in_=st["GT"][:, 128 * i : 128 * (i + 1)],
                            transpose=True,
                        )
                    for i in range(NT):
                        nc.vector.tensor_scalar_mul(
                            out=Gn[:, i, :], in0=gnu[:, i, :], scalar1=ac[:, i : i + 1]
                        )
                else:
                    trp = ps_mm.tile([128, NT, 128], BF16, tag="mm")
                    for i in range(NT):
                        nc.tensor.transpose(
                            trp[:, i, :], st["GT"][:, 128 * i : 128 * (i + 1)], ident
                        )
                    for i in range(NT):
                        if i % 2 == 0:
                            nc.scalar.mul(Gn[:, i, :], trp[:, i, :], ac[:, i : i + 1])
                        else:
                            nc.vector.tensor_scalar_mul(
                                out=Gn[:, i, :], in0=trp[:, i, :],
                                scalar1=ac[:, i : i + 1],
                            )
                st["Gn"] = Gn

            def stage_b(b, st):
                """t = S_T' @ [c | 1] (numerator + col-sum column) -> tB."""
                tps = ps_t.tile([128, D + 1], F32, tag="tps")
                for i in range(NT):
                    nc.tensor.matmul(
                        tps, st["Gn"][:, i, :], st["c_n1"][:, i, :],
                        start=(i == 0), stop=(i == NT - 1),
                    )
                csi = pb.tile([128, 1], F32, tag="csi")
                nc.vector.reciprocal(csi, tps[:, D : D + 1])
                bqc = pb.tile([128, 1], F32, tag="bqc")
                nc.vector.tensor_mul(bqc, lb_all[:, b, NT : NT + 1], csi)
                tB = pb.tile([128, D + 1], BF16, tag="tB")
                nc.scalar.mul(tB, tps[:, 0 : D + 1], bqc)
                st["tB"] = tB

            def stage_c(b, st):
                """c2q/q2c matmuls + strided evacuation + store.
                Output stays denominator-scaled; host divides."""
                obuf = po.tile([128, NT, 2 * (D + 1)], BF16, tag="obuf")
                for i in range(NT):
                    big2 = ps_o.tile([128, 1024], F32, tag="big2")
                    gsl = st["GT"][:, 128 * i : 128 * (i + 1)]
                    nc.tensor.matmul(
                        big2[:, 0 : D + 1], gsl, st["qbx"], start=True, stop=True
                    )
                    nc.tensor.matmul(
                        big2[:, 512 : 512 + D + 1], gsl, st["tB"], start=True, stop=True
                    )
                    src = big2.rearrange("p (g x) -> p g x", g=2)[:, :, 0 : D + 1]
                    dst = obuf[:, i, :].rearrange("p (g x) -> p g x", g=2)
                    if GN_VIA != "snat" and i % 2 == 0:
                        nc.scalar.copy(dst, src)
                    else:
                        nc.vector.tensor_copy(dst, src)
                nc.sync.dma_start(out=o_d[b], in_=obuf)

            # software pipeline, stage_a1 runs 2 batches ahead:
            #   iter b: a1(b+2) | a2(b+1) | t(b) + out(b)
            sts = {}
            sts[0] = stage_a1(0)
            if bpc > 1:
                sts[1] = stage_a1(1)
            stage_a2(0, sts[0])
            for b in range(bpc):
                if b + 2 < bpc:
                    sts[b + 2] = stage_a1(b + 2)
                if b + 1 < bpc:
                    stage_a2(b + 1, sts[b + 1])
                stage_b(b, sts[b])
                stage_c(b, sts[b])
                del sts[b]

    nc.finalize()
    return nc


_NC = None


def _get_nc():
    global _NC
    if _NC is None:
        _NC = _build()
    return _NC


def _in_maps(inputs):
    """Host-side staging: shard over batch, cast to bf16, pre-stage each
    tensor in the exact SBUF layout the kernel DMAs it into, and
    precompute the per-row/per-col softmax scale vectors."""
    c = np.ascontiguousarray(inputs["c"], dtype=np.float32)
    q = np.ascontiguousarray(inputs["q"], dtype=np.float32)
    w0 = np.asarray(inputs["w0"], dtype=np.float32).reshape(D)
    w1 = np.asarray(inputs["w1"], dtype=np.float32).reshape(D)
    wm = np.asarray(inputs["wm"], dtype=np.float32).reshape(D)
    bias = np.asarray(inputs["bias"], dtype=np.float32).reshape(M)
    cmf = np.asarray(inputs["c_mask"], dtype=np.float32)  # [B, N]
    qmf = np.asarray(inputs["q_mask"], dtype=np.float32)  # [B, M]

    cb = c.astype(BF)
    # c natural with ones column: [B, 128, NT, D+1]
    c_st = np.ones((B, 128, NT, D + 1), dtype=BF)
    c_st[:, :, :, 0:D] = cb.reshape(B, NT, 128, D).transpose(0, 2, 1, 3)
    # c^T: [B, 128, DT, N]
    ct_st = np.ascontiguousarray(
        cb.transpose(2, 0, 1).reshape(DT, 128, B, N).transpose(2, 1, 0, 3)
    )
    # qside: [B, 128, 513] = [(q*wm)^T as DTx128 | q*Bq | Bq]
    bq = np.exp(q @ w1 + bias[None, :]) * qmf  # [B, M]
    qs_st = np.empty((B, 128, 513), dtype=BF)
    qwm = (q * wm[None, None, :]).astype(BF)  # [B, M, D]
    qs_st[:, :, 0:256] = (
        qwm.transpose(2, 0, 1).reshape(DT, 128, B, M).transpose(2, 1, 0, 3)
        .reshape(B, 128, 256)
    )
    qs_st[:, :, 256:512] = (q * bq[:, :, None]).astype(BF)
    qs_st[:, :, 512] = bq.astype(BF)
    # lacbq: [128, B, NT+1]
    lb_st = np.empty((128, B, NT + 1), dtype=np.float32)
    lac = c @ w0 + np.where(cmf > 0.5, 0.0, -1e4)  # [B, N]
    lb_st[:, :, 0:NT] = lac.reshape(B, NT, 128).transpose(2, 0, 1)
    lb_st[:, :, NT] = bq.transpose(1, 0)

    in_maps = []
    for k in range(NCORES):
        s = slice(k * BPC, (k + 1) * BPC)
        in_maps.append(
            {
                "c": np.ascontiguousarray(c_st[s]),
                "ct": np.ascontiguousarray(ct_st[s]),
                "qs": np.ascontiguousarray(qs_st[s]),
                "lb": np.ascontiguousarray(lb_st[:, s]),
            }
        )
    return in_maps


def kernel(c, q, c_mask, q_mask, w0, w1, wm, bias):
    inputs = {
        "c": c, "q": q, "c_mask": c_mask, "q_mask": q_mask,
        "w0": w0, "w1": w1, "wm": wm, "bias": bias,
    }
    in_maps = _in_maps(inputs)
    res = run_bass_kernel_spmd(_get_nc(), in_maps, core_ids=list(range(NCORES)))
    dev = np.concatenate(
        [np.asarray(res.results[k]["out"]) for k in range(NCORES)], axis=0
    )  # [B, 128, NT, 2*(D+1)]
    dev = dev.transpose(0, 2, 1, 3).reshape(B, N, 2 * (D + 1)).astype(np.float32)
    den = dev[:, :, D : D + 1]  # (G @ Bq)[n], denominator-scaled path
    c2q = dev[:, :, 0:D] / den
    q2c = dev[:, :, D + 1 : 2 * D + 1] / den
    c32 = np.ascontiguousarray(c, dtype=np.float32)
    out = np.empty((B, N, 4 * D), dtype=np.float32)
    out[:, :, 0:D] = c32
    out[:, :, D : 2 * D] = c2q
    out[:, :, 2 * D : 3 * D] = c32 * c2q
    out[:, :, 3 * D : 4 * D] = c32 * q2c
    return out


# revision 14
# speedup vs baseline: 1.3749x; 1.0384x over previous
"""Trainium2 Bass kernel for ContextQueryAttention (BiDAF-style), v5.

Math (per batch):
    S[n,m] = c@w0 [n] + (q@w1 + bias)[m] + sum_d c[n,d]*wm[d]*q[m,d]
    S_  = softmax_m(S + MASK*(1-q_mask))          # row softmax
    S_T = softmax_n(S + MASK*(1-c_mask)).T        # col softmax, transposed
    c2q = S_ @ q ;  q2c = S_ @ (S_T @ c)
    out = [c | c2q | c*c2q | c*q2c]

Factorization: with G = exp(sub2), A = exp(sub0), B = exp(sub1+bias),
    S_[n,m]  = G[n,m]*Bq[m] / (G @ Bq)[n]         Bq = B * q_mask
    S_T[m,n] = G[n,m]*Ac[n] / (G.T @ Ac)[m]       Ac = A * c_mask

v5 (vs v4, 91 us):
  - Gn via xbar DMA-transpose of GT (sync queue) instead of PE
    transposes: PE drops from 36 to 28 matmuls/batch (GN_VIA switch
    falls back to the PE path).
  - device output is UNNORMALIZED [c2q*den | den-col | q2c*den]; the
    row denominator rides along and the host divides, removing the 8
    per-tile reciprocal ops; evacuation is one strided plain copy/tile.
  - per-batch staggered loads (cT/qside per batch) instead of one 4 MB
    up-front global, so the pipeline fills in ~2 us instead of ~14 us.
  - output stores issue from the scalar queue (separate HWDGE ring from
    the loads).
Sharding: data-parallel over batch, 8 batches per core on 8 cores.
"""

import sys

if "/opt/trn_rl_repo" not in sys.path:
    sys.path.insert(0, "/opt/trn_rl_repo")

import numpy as np
import ml_dtypes

import concourse.bass as bass
import concourse.mybir as mybir
import concourse.tile as tile
from concourse import bacc
from concourse.bass_utils import run_bass_kernel_spmd
from concourse.masks import make_identity

B, N, M, D = 64, 1024, 128, 256
NCORES = 8
BPC = B // NCORES  # batches per core
NT = N // 128      # n-tiles per batch
DT = D // 128      # d-tiles

F32 = mybir.dt.float32
BF16 = mybir.dt.bfloat16
EXP = mybir.ActivationFunctionType.Exp
X = mybir.AxisListType.X
BF = ml_dtypes.bfloat16

GN_VIA = "pe"  # "dma" | "pe" | "snat" (recompute S in natural layout)


def _build(bpc: int = BPC):
    nc = bacc.Bacc(trn_type="TRN2")

    # all staged by the host in device/SBUF layout so every DMA is a
    # contiguous <=3-dim AP with multi-KB per-partition lines
    c_d = nc.dram_tensor("c", [bpc, 128, NT, D + 1], BF16, kind="ExternalInput")
    ct_d = nc.dram_tensor("ct", [bpc, 128, DT, N], BF16, kind="ExternalInput")
    # qside: cols [0:256] = (q*wm)^T as [DT,128], cols [256:513] = [q*Bq | Bq]
    qs_d = nc.dram_tensor("qs", [bpc, 128, 513], BF16, kind="ExternalInput")
    # lacbq: [:, b, 0:NT] = c@w0 + log(c_mask); [:, b, NT] = Bq
    lb_d = nc.dram_tensor("lb", [128, bpc, NT + 1], F32, kind="ExternalInput")
    o_d = nc.dram_tensor("out", [bpc, 128, NT, 2 * (D + 1)], BF16, kind="ExternalOutput")

    with tile.TileContext(nc) as tc:
        with (
            tc.tile_pool(name="glob", bufs=1) as gp,
            tc.tile_pool(name="pa", bufs=3) as pa,
            tc.tile_pool(name="pb", bufs=2) as pb,
            tc.tile_pool(name="po", bufs=2) as po,
            tc.tile_pool(name="ps_mm", bufs=3, space="PSUM") as ps_mm,
            tc.tile_pool(name="ps_t", bufs=1, space="PSUM") as ps_t,
            tc.tile_pool(name="ps_o", bufs=2, space="PSUM") as ps_o,
        ):
            # ---- globals ----
            if GN_VIA == "pe":
                ident = gp.tile([128, 128], BF16)
                make_identity(nc, ident)
            lb_all = gp.tile([128, bpc, NT + 1], F32)
            nc.sync.dma_start(out=lb_all, in_=lb_d[:, :, :])

            def stage_a1(b):
                """loads + S^T matmuls + GT exps + Ac exp."""
                st = {}
                qside = pa.tile([128, 513], BF16, tag="qside")
                nc.sync.dma_start(out=qside, in_=qs_d[b])
                cTb = pa.tile([128, DT, N], BF16, tag="cTb")
                nc.sync.dma_start(out=cTb, in_=ct_d[b])
                c_n1 = pa.tile([128, NT, D + 1], BF16, tag="c_n1")
                nc.sync.dma_start(out=c_n1, in_=c_d[b])

                GT = pa.tile([128, N], BF16, tag="GT")
                for h in range(2):
                    stp = ps_mm.tile([128, 512], F32, tag="mm")
                    for j in range(DT):
                        nc.tensor.matmul(
                            stp,
                            qside[:, 128 * j : 128 * (j + 1)],
                            cTb[:, j, 512 * h : 512 * (h + 1)],
                            start=(j == 0),
                            stop=(j == DT - 1),
                        )
                    nc.scalar.activation(GT[:, 512 * h : 512 * (h + 1)], stp, EXP)

                if GN_VIA != "snat":
                    ac = pa.tile([128, NT], F32, tag="ac")
                    nc.scalar.activation(ac, lb_all[:, b, 0:NT], EXP)
                    st["ac"] = ac
                st["c_n1"], st["GT"] = c_n1, GT
                st["qside"], st["cTb"] = qside, cTb
                st["qbx"] = qside[:, 256:513]
                return st

            def stage_a2(b, st):
                """Gn' = G * Ac in n-partition layout."""
                Gn = pa.tile([128, NT, M], BF16, tag="Gn")
                if GN_VIA == "snat":
                    # recompute S with n as the output partition; exp folds
                    # Ac in via the per-partition bias lac = c@w0 + log(cm)
                    qside, cTb = st["qside"], st["cTb"]
                    for g in range(2):
                        sn = ps_mm.tile([128, 512], F32, tag="mm")
                        for u in range(4):
                            i = 4 * g + u
                            for j in range(DT):
                                nc.tensor.matmul(
                                    sn[:, 128 * u : 128 * (u + 1)],
                                    cTb[:, j, 128 * i : 128 * (i + 1)],
                                    qside[:, 128 * j : 128 * (j + 1)],
                                    start=(j == 0),
                                    stop=(j == DT - 1),
                                )
                        for u in range(4):
                            i = 4 * g + u
                            nc.scalar.activation(
                                Gn[:, i, :], sn[:, 128 * u : 128 * (u + 1)], EXP,
                                bias=lb_all[:, b, i : i + 1], scale=1.0,
                            )
                    st["Gn"] = Gn
                    return
                ac = st["ac"]
                if GN_VIA == "dma":
                    gnu = pa.tile([128, NT, M], BF16, tag="gnu")
                    for i in range(NT):
                        nc.sync.dma_start(
                            out=gnu[:, i, :],
                            in_=st["GT"][:, 128 * i : 128 * (i + 1)],
                            transpose=True,
                        )
                    for i in range(NT):
                        nc.vector.tensor_scalar_mul(
                            out=Gn[:, i, :], in0=gnu[:, i, :], scalar1=ac[:, i : i + 1]
                        )
                else:
                    trp = ps_mm.tile([128, NT, 128], BF16, tag="mm")
                    for i in range(NT):
                        nc.tensor.transpose(
                            trp[:, i, :], st["GT"][:, 128 * i : 128 * (i + 1)], ident
                        )
                    for i in range(NT):
                        if i % 2 == 0:
                            nc.scalar.mul(Gn[:, i, :], trp[:, i, :], ac[:, i : i + 1])
                        else:
                            nc.vector.tensor_scalar_mul(
                                out=Gn[:, i, :], in0=trp[:, i, :],
                                scalar1=ac[:, i : i + 1],
                            )
                st["Gn"] = Gn

            def stage_b(b, st):
                """t = S_T' @ [c | 1] (numerator + col-sum column) -> tB."""
                tps = ps_t.tile([128, D + 1], F32, tag="tps")
                for i in range(NT):
                    nc.tensor.matmul(
                        tps, st["Gn"][:, i, :], st["c_n1"][:, i, :],
                        start=(i == 0), stop=(i == NT - 1),
                    )
                csi = pb.tile([128, 1], F32, tag="csi")
                nc.vector.reciprocal(csi, tps[:, D : D + 1])
                bqc = pb.tile([128, 1], F32, tag="bqc")
                nc.vector.tensor_mul(bqc, lb_all[:, b, NT : NT + 1], csi)
                tB = pb.tile([128, D + 1], BF16, tag="tB")
                nc.scalar.mul(tB, tps[:, 0 : D + 1], bqc)
                st["tB"] = tB

            def stage_c(b, st):
                """c2q/q2c matmuls + strided evacuation + store.
                Output stays denominator-scaled; host divides."""
                obuf = po.tile([128, NT, 2 * (D + 1)], BF16, tag="obuf")
                for i in range(NT):
                    big2 = ps_o.tile([128, 1024], F32, tag="big2")
                    gsl = st["GT"][:, 128 * i : 128 * (i + 1)]
                    nc.tensor.matmul(
                        big2[:, 0 : D + 1], gsl, st["qbx"], start=True, stop=True
                    )
                    nc.tensor.matmul(
                        big2[:, 512 : 512 + D + 1], gsl, st["tB"], start=True, stop=True
                    )
                    src = big2.rearrange("p (g x) -> p g x", g=2)[:, :, 0 : D + 1]
                    dst = obuf[:, i, :].rearrange("p (g x) -> p g x", g=2)
                    if GN_VIA != "snat" and i % 2 == 0:
                        nc.scalar.copy(dst, src)
                    else:
                        nc.vector.tensor_copy(dst, src)
                nc.sync.dma_start(out=o_d[b], in_=obuf)

            # software pipeline, stage_a1 runs 2 batches ahead:
            #   iter b: a1(b+2) | a2(b+1) | t(b) + out(b)
            sts = {}
            sts[0] = stage_a1(0)
            if bpc > 1:
                sts[1] = stage_a1(1)
            stage_a2(0, sts[0])
            for b in range(bpc):
                if b + 2 < bpc:
                    sts[b + 2] = stage_a1(b + 2)
                if b + 1 < bpc:
                    stage_a2(b + 1, sts[b + 1])
                stage_b(b, sts[b])
                stage_c(b, sts[b])
                del sts[b]

    nc.finalize()
    return nc


_NC = None


def _get_nc():
    global _NC
    if _NC is None:
        _NC = _build()
    return _NC


def _in_maps(inputs):
    """Host-side staging: shard over batch, cast to bf16, pre-stage each
    tensor in the exact SBUF layout the kernel DMAs it into, and
    precompute the per-row/per-col softmax scale vectors."""
    c = np.ascontiguousarray(inputs["c"], dtype=np.float32)
    q = np.ascontiguousarray(inputs["q"], dtype=np.float32)
    w0 = np.asarray(inputs["w0"], dtype=np.float32).reshape(D)
    w1 = np.asarray(inputs["w1"], dtype=np.float32).reshape(D)
    wm = np.asarray(inputs["wm"], dtype=np.float32).reshape(D)
    bias = np.asarray(inputs["bias"], dtype=np.float32).reshape(M)
    cmf = np.asarray(inputs["c_mask"], dtype=np.float32)  # [B, N]
    qmf = np.asarray(inputs["q_mask"], dtype=np.float32)  # [B, M]

    cb = c.astype(BF)
    # c natural with ones column: [B, 128, NT, D+1]
    c_st = np.ones((B, 128, NT, D + 1), dtype=BF)
    c_st[:, :, :, 0:D] = cb.reshape(B, NT, 128, D).transpose(0, 2, 1, 3)
    # c^T: [B, 128, DT, N]
    ct_st = np.ascontiguousarray(
        cb.transpose(2, 0, 1).reshape(DT, 128, B, N).transpose(2, 1, 0, 3)
    )
    # qside: [B, 128, 513] = [(q*wm)^T as DTx128 | q*Bq | Bq]
    bq = np.exp(q @ w1 + bias[None, :]) * qmf  # [B, M]
    qs_st = np.empty((B, 128, 513), dtype=BF)
    qwm = (q * wm[None, None, :]).astype(BF)  # [B, M, D]
    qs_st[:, :, 0:256] = (
        qwm.transpose(2, 0, 1).reshape(DT, 128, B, M).transpose(2, 1, 0, 3)
        .reshape(B, 128, 256)
    )
    qs_st[:, :, 256:512] = (q * bq[:, :, None]).astype(BF)
    qs_st[:, :, 512] = bq.astype(BF)
    # lacbq: [128, B, NT+1]
    lb_st = np.empty((128, B, NT + 1), dtype=np.float32)
    lac = c @ w0 + np.where(cmf > 0.5, 0.0, -1e4)  # [B, N]
    lb_st[:, :, 0:NT] = lac.reshape(B, NT, 128).transpose(2, 0, 1)
    lb_st[:, :, NT] = bq.transpose(1, 0)

    in_maps = []
    for k in range(NCORES):
        s = slice(k * BPC, (k + 1) * BPC)
        in_maps.append(
            {
                "c": np.ascontiguousarray(c_st[s]),
                "ct": np.ascontiguousarray(ct_st[s]),
                "qs": np.ascontiguousarray(qs_st[s]),
                "lb": np.ascontiguousarray(lb_st[:, s]),
            }
        )
    return in_maps


def kernel(c, q, c_mask, q_mask, w0, w1, wm, bias):
    inputs = {
        "c": c, "q": q, "c_mask": c_mask, "q_mask": q_mask,
        "w0": w0, "w1": w1, "wm": wm, "bias": bias,
    }
    in_maps = _in_maps(inputs)
    res = run_bass_kernel_spmd(_get_nc(), in_maps, core_ids=list(range(NCORES)))
    dev = np.concatenate(
        [np.asarray(res.results[k]["out"]) for k in range(NCORES)], axis=0
    )  # [B, 128, NT, 2*(D+1)]
    dev = dev.transpose(0, 2, 1, 3).reshape(B, N, 2 * (D + 1)).astype(np.float32)
    den = dev[:, :, D : D + 1]  # (G @ Bq)[n], denominator-scaled path
    c2q = dev[:, :, 0:D] / den
    q2c = dev[:, :, D + 1 : 2 * D + 1] / den
    c32 = np.ascontiguousarray(c, dtype=np.float32)
    out = np.empty((B, N, 4 * D), dtype=np.float32)
    out[:, :, 0:D] = c32
    out[:, :, D : 2 * D] = c2q
    out[:, :, 2 * D : 3 * D] = c32 * c2q
    out[:, :, 3 * D : 4 * D] = c32 * q2c
    return out
